# revision 36
# baseline (speedup 1.0000x reference)
"""Multi-head self-attention Bass/Tile kernel for TRN2.

Per-core problem (batch sharded across 8 cores):
  x [N=1024, C=768], Wqkv [768, 2304], bqkv [2304], Wproj [768, 768], bproj [768]
  -> y [1024, 768]

Layout strategy (cost model charges matmuls by OUTPUT FREE SIZE only):
  xT   [C, N]      via PE transposes of x (bf16 identity -> 1 cyc/row)
  qkT  [2C, N]     = W.T-matmul with rhs xT (fp32r production), stored bf16
  V    [N, C]      xT-stationary matmul, stored bf16 padded per head with a
                   ones column (65 cols/head) for softmax sums
  S^T  [keys, n]   per head = kT.T @ qT  (K = 64, PE row groups alternate)
  P^T  = exp(S^T * 0.125)  (ACT, direct from PSUM, written bf16)
  O    [q, 65]     per query-tile = P^T-chunk.T @ [V_h | 1] accumulated over
                   key-tiles; col 64 = softmax sums.  Charged only 65 PE
                   rows/matmul (vs 512 in the [65, n] orientation).
  oc   [q, f]      = O[:, 0:64] * (1/O[:, 64]) per-partition scalars (DVE)
  aoT  [C, N]      = DMA crossbar transpose of oc (no PE/DVE cost)
  y    [N, C]      = aoT.T-matmul with rhs Wproj (bf16), accumulated into
                   SBUF in k-groups so most proj work overlaps attention

Scheduling: the attention inner loop is software-pipelined (PV for slot j-1
issued after S/exp of slot j) so the in-order PE queue never blocks on an
unfinished exp; qkT/V/proj production is split into fine-grained filler
closures pumped into the ~400ns PE gaps of the ACT-paced loop.
"""

import numpy as np

N = 1024
C = 768
H = 12
D = 64
NT = N // 128   # 8 token tiles
CT = C // 128   # 6 channel tiles
MT_QK = 2 * C // 128  # 12 feature tiles for q|k
PAIRS = H // 2  # 6
SCALE = 1.0 / np.sqrt(D)


def build_nc(mm_dtype_name="float32r"):
    import ml_dtypes
    import concourse.bass as bass
    import concourse.tile as tile
    from concourse import bacc, mybir
    from collections import deque

    mm_dt = getattr(mybir.dt, mm_dtype_name)
    f32 = mybir.dt.float32
    bf16 = mybir.dt.bfloat16

    nc = bacc.Bacc(None, target_bir_lowering=False)

    x = nc.dram_tensor("x", [N, C], f32, kind="ExternalInput")
    wqkv = nc.dram_tensor("Wqkv", [C, 3 * C], f32, kind="ExternalInput")
    bqkv = nc.dram_tensor("bqkv", [3 * C], f32, kind="ExternalInput")
    wproj = nc.dram_tensor("Wproj", [C, C], f32, kind="ExternalInput")
    bproj = nc.dram_tensor("bproj", [C], f32, kind="ExternalInput")
    y = nc.dram_tensor("y", [N, C], f32, kind="ExternalOutput")

    mm = nc.tensor.matmul

    with tile.TileContext(nc) as tc:
        with (
            tc.tile_pool(name="const", bufs=1) as const,
            tc.tile_pool(name="xt", bufs=1) as xt_pool,
            tc.tile_pool(name="qk", bufs=8) as qk_pool,
            tc.tile_pool(name="vpad", bufs=NT) as vpad_pool,
            tc.tile_pool(name="aot", bufs=CT) as aot_pool,
            tc.tile_pool(name="ocq", bufs=2) as oc_pool,
            tc.tile_pool(name="xin", bufs=4) as x_pool,
            tc.tile_pool(name="wqk", bufs=4) as wqk_pool,
            tc.tile_pool(name="wrhs", bufs=2 * CT) as wrhs_pool,
            tc.tile_pool(name="wstage", bufs=2) as wstage_pool,
            tc.tile_pool(name="pexp", bufs=4) as p_pool,
            tc.tile_pool(name="inv", bufs=2) as inv_pool,
            tc.tile_pool(name="yacc", bufs=NT) as yacc_pool,
            tc.tile_pool(name="yout", bufs=6) as y_pool,
            tc.tile_pool(name="psS", bufs=3, space="PSUM") as psS,
            tc.tile_pool(name="psO", bufs=2, space="PSUM") as psO,
        ):
            # ---- constants; ident first (transposes need it early) ----
            ident_dram = nc.inline_tensor(
                np.eye(128, dtype=ml_dtypes.bfloat16), name="ident128"
            )
            ident = const.tile([128, 128], bf16)
            nc.sync.dma_start(ident[:], ident_dram.ap())
            bq_cols = const.tile([128, MT_QK], f32)
            nc.gpsimd.dma_start(
                bq_cols[:], bqkv.ap().rearrange("(m p) -> p m", p=128)[:, 0:MT_QK]
            )
            bqv_row = const.tile([1, C], f32)
            nc.gpsimd.dma_start(bqv_row[:], bqkv.ap()[None, 2 * C : 3 * C])
            bqv_bc = const.tile([128, C], f32)
            nc.gpsimd.partition_broadcast(bqv_bc[:], bqv_row[:])
            bp_row = const.tile([1, C], f32)
            nc.gpsimd.dma_start(bp_row[:], bproj.ap()[None, :])
            bp_bc = const.tile([128, C], f32)
            nc.gpsimd.partition_broadcast(bp_bc[:], bp_row[:])
            ones_c = const.tile([128, H], f32)
            nc.vector.memset(ones_c[:], 1.0)

            # ---- upfront DMAs on the two HWDGE queues (SP + ACT); the
            # gpsimd/SWDGE queue is kept for consts + Wproj so x tiles are
            # never stuck behind slow Pool descriptor generation ----
            xi_t = []
            for i in range(NT):
                xi = x_pool.tile([128, C], f32, tag="xi", name=f"xi{i}")
                xi_t.append(xi)

            wqk_tiles = {}

            def wqk_dma(mi, eng):
                # one DMA per M-tile: [128, CT*128], col = k*128 + m
                w = wqk_pool.tile([128, C], mm_dt, name=f"wqk{mi}", tag="w")
                eng.dma_start(
                    w[:].bitcast(f32).rearrange("p (k m) -> p k m", k=CT),
                    wqkv.ap()[:, mi * 128 : (mi + 1) * 128].rearrange(
                        "(k p) m -> p k m", p=128
                    ),
                )
                wqk_tiles[mi] = w

            wv_tiles = [
                wrhs_pool.tile([128, C], mm_dt, name=f"wv{k}", tag="wrhs")
                for k in range(CT)
            ]

            def wv_dma(k, eng):
                eng.dma_start(
                    wv_tiles[k][:].bitcast(f32),
                    wqkv.ap()[k * 128 : (k + 1) * 128, 2 * C : 3 * C],
                )

            # arrival order on the shared DMA device tracks issue order:
            # x0-x3 + the two first-pair W tiles come first
            nc.sync.dma_start(xi_t[0][:], x.ap()[0:128, :])
            nc.scalar.dma_start(xi_t[1][:], x.ap()[128:256, :])
            nc.sync.dma_start(xi_t[2][:], x.ap()[256:384, :])
            nc.scalar.dma_start(xi_t[3][:], x.ap()[384:512, :])
            wqk_dma(0, nc.sync)
            wqk_dma(PAIRS, nc.scalar)
            nc.sync.dma_start(xi_t[4][:], x.ap()[512:640, :])
            nc.scalar.dma_start(xi_t[5][:], x.ap()[640:768, :])
            nc.sync.dma_start(xi_t[6][:], x.ap()[768:896, :])
            nc.scalar.dma_start(xi_t[7][:], x.ap()[896:1024, :])
            for k in range(CT):
                wv_dma(k, nc.sync if k % 2 == 0 else nc.scalar)

            # Wproj staged + cast to bf16 on Pool (needed from pair 3 on)
            wp_tiles = []
            for k in range(CT):
                stage = wstage_pool.tile([128, C], f32, name=f"wps{k}", tag="wstage")
                nc.gpsimd.dma_start(stage[:], wproj.ap()[k * 128 : (k + 1) * 128, :])
                wp = wrhs_pool.tile([128, C], bf16, name=f"wp{k}", tag="wrhs")
                nc.gpsimd.tensor_copy(wp[:], stage[:])
                wp_tiles.append(wp)

            # ---- xT via PE transposes (bf16 identity: 1 cyc/row) ----
            xt_all = xt_pool.tile([128, CT * N], mm_dt)
            xt3 = xt_all[:].rearrange("p (k n) -> p k n", k=CT)

            def xt_k(k):
                return xt_all[:, k * N : (k + 1) * N]

            for i in range(NT):
                pt = psS.tile([128, C], f32, tag="s", name=f"pt{i}")
                for j in range(CT):
                    nc.tensor.transpose(
                        pt[:, j * 128 : (j + 1) * 128].bitcast(mm_dt),
                        xi_t[i][:, j * 128 : (j + 1) * 128].bitcast(mm_dt),
                        ident[:],
                    )
                nc.vector.tensor_copy(
                    xt3[:, :, i * 128 : (i + 1) * 128],
                    pt[:].rearrange("p (k b) -> p k b", k=CT),
                )

            # ---- qkT production (bf16 storage, fp32r compute) ----
            qkT = [
                qk_pool.tile([128, N], bf16, name=f"qkT{m}", tag="qkT")
                for m in range(MT_QK)
            ]
            qk_ps = {}

            def qkT_mm(mi, c0, k):
                w = wqk_tiles[mi]
                if k == 0:
                    qk_ps[(mi, c0)] = psS.tile(
                        [128, 512], f32, tag="s", name=f"qps{mi}_{c0}"
                    )
                mm(qk_ps[(mi, c0)][:], w[:, k * 128 : (k + 1) * 128],
                   xt_k(k)[:, c0 : c0 + 512],
                   start=(k == 0), stop=(k == CT - 1))

            def qkT_evict(mi, c0):
                nc.vector.tensor_scalar_add(
                    qkT[mi][:, c0 : c0 + 512], qk_ps[(mi, c0)][:],
                    bq_cols[:, mi : mi + 1],
                )

            def qkT_chunk(mi, c0):
                for k in range(CT):
                    qkT_mm(mi, c0, k)
                qkT_evict(mi, c0)

            # pair-0 qkT inline, ordered so S(h0, j0) is ready earliest
            qkT_chunk(0, 0)
            qkT_chunk(PAIRS, 0)
            qkT_chunk(0, 512)
            qkT_chunk(PAIRS, 512)

            # ---- V (bf16, padded with ones col per head) ----
            vpad = [
                vpad_pool.tile([128, H * (D + 1)], bf16, name=f"vpad{t}", tag="vpad")
                for t in range(NT)
            ]
            v_ps = {}

            def V_part(ti, part):
                # parts 0-2: two k-steps each; part 3: bias add + ones col
                if part == 0:
                    v_ps[ti] = psS.tile([128, C], f32, tag="s", name=f"vps{ti}")
                ps = v_ps[ti]
                if part < 3:
                    for k in (2 * part, 2 * part + 1):
                        lhsT = xt_k(k)[:, ti * 128 : (ti + 1) * 128]
                        mm(ps[:, 0:512], lhsT, wv_tiles[k][:, 0:512],
                           start=(k == 0), stop=(k == CT - 1))
                        mm(ps[:, 512:768], lhsT, wv_tiles[k][:, 512:768],
                           start=(k == 0), stop=(k == CT - 1))
                else:
                    vsl = vpad[ti][:].rearrange("p (h d) -> p h d", h=H)
                    nc.vector.tensor_tensor(
                        out=vsl[:, :, 0:D],
                        in0=ps[:].rearrange("p (h d) -> p h d", h=H),
                        in1=bqv_bc[:].rearrange("p (h d) -> p h d", h=H),
                        op=mybir.AluOpType.add,
                    )
                    nc.vector.tensor_copy(vsl[:, :, D], ones_c[:])

            # ---- proj accumulators (SBUF) + k-group chains ----
            yacc = [
                yacc_pool.tile([128, C], f32, name=f"yacc{t}", tag="yacc")
                for t in range(NT)
            ]
            pj_ps = {}

            def proj_k(ti, k, first, last):
                if first:
                    pj_ps[ti] = psS.tile([128, C], f32, tag="s", name=f"pj{ti}_{k}")
                ps = pj_ps[ti]
                lhsT = aot[k][:, ti * 128 : (ti + 1) * 128]
                mm(ps[:, 0:512], lhsT, wp_tiles[k][:, 0:512],
                   start=first, stop=last)
                mm(ps[:, 512:768], lhsT, wp_tiles[k][:, 512:768],
                   start=first, stop=last)

            def proj_combine(ti, first):
                ps = pj_ps[ti]
                nc.vector.tensor_tensor(
                    out=yacc[ti][:],
                    in0=ps[:],
                    in1=(bp_bc[:] if first else yacc[ti][:]),
                    op=mybir.AluOpType.add,
                )

            # ---- attention structures ----
            aot = [
                aot_pool.tile([128, N], bf16, name=f"aot{j}", tag="aot")
                for j in range(CT)
            ]

            fill_q = deque()

            def pump(budget):
                while fill_q and budget > 0:
                    rows, fn = fill_q.popleft()
                    fn()
                    budget -= rows

            def enqueue_pair(pp):
                # weight DMAs issue immediately (a full pair of lead time);
                # only the fine-grained matmul/evict closures are pumped
                for mi in (pp, PAIRS + pp):
                    # pair 1 is enqueued at startup while gpsimd is clogged
                    # with consts/Wproj: use the SP HWDGE queue for both
                    eng = nc.sync if (mi == pp or pp == 1) else nc.gpsimd
                    wqk_dma(mi, eng)
                    for c0 in (0, 512):
                        for k in range(CT):
                            fill_q.append(
                                (512, lambda mi=mi, c0=c0, k=k: qkT_mm(mi, c0, k))
                            )
                        fill_q.append(
                            (0, lambda mi=mi, c0=c0: qkT_evict(mi, c0))
                        )

            def enqueue_proj(ks, first_k, tis):
                for ti in tis:
                    for idx, k in enumerate(ks):
                        fill_q.append(
                            (768,
                             lambda ti=ti, k=k, idx=idx, n=len(ks): proj_k(
                                 ti, k, idx == 0, idx == n - 1))
                        )
                    fill_q.append(
                        (0, lambda ti=ti, fk=first_k: proj_combine(ti, fk))
                    )

            enqueue_pair(1)

            # ---- software-pipelined attention loop ----
            # state for the one-slot-delayed PV + end-of-head normalize
            pend = None  # (h, j, P, Oc, vsl_off) for PV not yet issued

            def issue_pv(h, j, P, Oc):
                for qt in range(NT):
                    ci, q4 = divmod(qt, 4)
                    mm(Oc[ci][:, q4 * 65 : (q4 + 1) * 65],
                       P[:, qt * 128 : (qt + 1) * 128],
                       vpad[j][:, h * (D + 1) : (h + 1) * (D + 1)],
                       start=(j == 0 and q4 == 0),
                       stop=(j == NT - 1 and q4 == 3))

            def normalize(h, Oc):
                # per-head half tile [q, qt*64+d]; transposed immediately so
                # only the last head's 448ns transpose sits in the tail
                p, h2 = divmod(h, 2)
                inv = inv_pool.tile([128, NT], f32, tag="inv", name=f"inv{h}")
                occ = oc_pool.tile([128, NT * D], bf16, name=f"oc{h}", tag="oc")
                for ci in range(2):
                    o3 = Oc[ci][:, 0:260].rearrange("p (q f) -> p q f", f=65)
                    nc.vector.reciprocal(inv[:, ci * 4 : (ci + 1) * 4], o3[:, :, D])
                    out3 = occ[:].rearrange("p (q f) -> p q f", f=D)
                    nc.vector.tensor_tensor(
                        out=out3[:, ci * 4 : (ci + 1) * 4, :],
                        in0=o3[:, :, 0:D],
                        in1=inv[:, ci * 4 : (ci + 1) * 4]
                        .unsqueeze(2)
                        .broadcast_to([128, 4, D]),
                        op=mybir.AluOpType.mult,
                    )
                # crossbar-transpose this head's 64 feature rows into aoT
                out3t = aot[p][h2 * D : (h2 + 1) * D, :].rearrange(
                    "p (q t) -> p q t", q=NT
                )
                if h == H - 1:
                    # final head: transpose each 4-qt half as soon as its
                    # normalize lands -- this is the tail's critical path
                    for ci in range(2):
                        nc.sync.dma_start_transpose(
                            out3t[:, ci * 4 : (ci + 1) * 4, :],
                            occ[:, ci * 4 * D : (ci + 1) * 4 * D],
                        )
                else:
                    nc.sync.dma_start_transpose(out3t, occ[:])

            for p in range(PAIRS):
                for h2 in range(2):
                    h = 2 * p + h2
                    hsl = slice(h2 * D, (h2 + 1) * D)
                    q_t = qkT[p]
                    k_t = qkT[PAIRS + p]
                    Oc = [
                        psO.tile([128, 512], f32, name=f"O{h}_{ci}", tag="o")
                        for ci in range(2)
                    ]
                    for j in range(NT):
                        S = psS.tile([128, N], f32, tag="s", name=f"S{h}_{j}")
                        mm(S[:, 0:512], k_t[hsl, j * 128 : (j + 1) * 128],
                           q_t[hsl, 0:512], start=True, stop=True,
                           tile_position=(h2 * D, 0))
                        mm(S[:, 512:1024], k_t[hsl, j * 128 : (j + 1) * 128],
                           q_t[hsl, 512:1024], start=True, stop=True,
                           tile_position=(h2 * D, 0))
                        P = p_pool.tile([128, N], bf16, tag="P", name=f"P{h}_{j}")
                        nc.scalar.activation(
                            P[:], S[:], mybir.ActivationFunctionType.Exp, scale=SCALE
                        )
                        # V production rides head 0 (PV is one slot behind)
                        if h == 0:
                            for part in range(4):
                                V_part(j, part)
                        # delayed PV from the previous slot
                        if pend is not None:
                            ph, pj, pP, pOc = pend
                            issue_pv(ph, pj, pP, pOc)
                            if pj == NT - 1:
                                normalize(ph, pOc)
                        pend = (h, j, P, Oc)
                        if p == PAIRS - 1 and h2 == 0 and j == 2:
                            # aot[4]'s transpose has landed by now
                            enqueue_proj((2, 3, 4), False, range(NT))
                        if h > 0:
                            # drain harder near the pair boundary so the next
                            # pair's first S is never stuck behind a burst
                            pump(2048 if (h2 == 1 and j >= 6) else 1024)
                # pair boundary: qkT for pair p+1 must be fully issued before
                # its first S matmul -> drain leftovers, then enqueue the next
                # batch of production work
                while fill_q:
                    fill_q.popleft()[1]()
                if p < PAIRS - 2:
                    enqueue_pair(p + 2)
                if p == 2:
                    # aot[0], aot[1] transposes landed during pair 2
                    enqueue_proj((0, 1), True, range(0, 4))
                elif p == 3:
                    enqueue_proj((0, 1), True, range(4, NT))

            # ---- tail: last PV + normalize + final proj k-group ----
            ph, pj, pP, pOc = pend
            issue_pv(ph, pj, pP, pOc)
            normalize(ph, pOc)
            for ti in range(NT):
                proj_k(ti, 5, True, True)
                ps = pj_ps[ti]
                yo = y_pool.tile([128, C], f32, tag="yo", name=f"yo{ti}")
                nc.vector.tensor_tensor(
                    out=yo[:], in0=ps[:], in1=yacc[ti][:], op=mybir.AluOpType.add
                )
                (nc.sync if ti % 2 == 0 else nc.scalar).dma_start(
                    y.ap()[ti * 128 : (ti + 1) * 128, :], yo[:]
                )

    nc.compile()
    return nc


_NC_CACHE = {}


def _get_nc(mm_dtype_name="float32r"):
    nc = _NC_CACHE.get(mm_dtype_name)
    if nc is None:
        nc = build_nc(mm_dtype_name)
        _NC_CACHE[mm_dtype_name] = nc
    return nc


_RUNNER_CACHE = {}
_DEV_CACHE = {}


def _get_runner(n_cores=8):
    """Cached jitted 8-core executor (PJRT path, no per-call retrace)."""
    if n_cores in _RUNNER_CACHE:
        return _RUNNER_CACHE[n_cores]
    import jax
    from jax.sharding import Mesh, PartitionSpec
    from jax.experimental.shard_map import shard_map
    from concourse import mybir
    from concourse.bass2jax import (
        _bass_exec_p,
        install_neuronx_cc_hook,
        partition_id_tensor,
    )

    nc = _get_nc()
    install_neuronx_cc_hook()
    partition_name = nc.partition_id_tensor.name if nc.partition_id_tensor else None

    in_names, out_names, out_avals = [], [], []
    for alloc in nc.m.functions[0].allocations:
        if not isinstance(alloc, mybir.MemoryLocationSet):
            continue
        name = alloc.memorylocations[0].name
        if alloc.kind == "ExternalInput":
            if name != partition_name:
                in_names.append(name)
        elif alloc.kind == "ExternalOutput":
            out_names.append(name)
            out_avals.append(
                jax.core.ShapedArray(
                    tuple(alloc.tensor_shape), mybir.dt.np(alloc.dtype)
                )
            )
    all_in_names = list(in_names)
    if partition_name is not None:
        all_in_names.append(partition_name)

    def _body(*args):
        operands = list(args)
        if partition_name is not None:
            operands.append(partition_id_tensor())
        return tuple(
            _bass_exec_p.bind(
                *operands,
                out_avals=tuple(out_avals),
                in_names=tuple(all_in_names),
                out_names=tuple(out_names),
                lowering_input_output_aliases=(),
                sim_require_finite=False,
                sim_require_nnan=False,
                nc=nc,
            )
        )

    devices = jax.devices()[:n_cores]
    mesh = Mesh(np.asarray(devices), ("core",))
    # x is batch-sharded; weights/biases are replicated (shipped once, not
    # 8x-concatenated on the host).
    in_specs = tuple(
        PartitionSpec("core") if n == "x" else PartitionSpec() for n in in_names
    )
    fn = jax.jit(
        shard_map(
            _body,
            mesh=mesh,
            in_specs=in_specs,
            out_specs=(PartitionSpec("core"),) * len(out_names),
            check_rep=False,
        ),
        keep_unused=True,
    )
    _RUNNER_CACHE[n_cores] = (fn, in_names, mesh)
    return _RUNNER_CACHE[n_cores]


def kernel(x, Wqkv, bqkv, Wproj, bproj):
    """Full-input entry point.

    x [8, 1024, 768] is sharded one batch element per NeuronCore (data
    parallel, weights replicated, no collectives); outputs are re-stacked.
    """
    x = np.ascontiguousarray(np.asarray(x, dtype=np.float32))
    Wqkv = np.ascontiguousarray(np.asarray(Wqkv, dtype=np.float32))
    bqkv = np.ascontiguousarray(np.asarray(bqkv, dtype=np.float32))
    Wproj = np.ascontiguousarray(np.asarray(Wproj, dtype=np.float32))
    bproj = np.ascontiguousarray(np.asarray(bproj, dtype=np.float32))
    B = x.shape[0]
    assert x.shape == (8, N, C), f"expected (8, {N}, {C}), got {x.shape}"

    arrays = {
        "x": x.reshape(B * N, C),
        "Wqkv": Wqkv,
        "bqkv": bqkv,
        "Wproj": Wproj,
        "bproj": bproj,
    }
    try:
        import jax
        from jax.sharding import NamedSharding, PartitionSpec

        fn, in_names, mesh = _get_runner(B)
        ops = []
        for n in in_names:
            a = arrays[n]
            if n == "x":
                ops.append(a)  # sharded fresh each call
                continue
            # weights rarely change call-to-call: keep them device-resident
            key = (n, id(a), a.shape)
            cached = _DEV_CACHE.get(n)
            if cached is None or cached[0] != key:
                dev = jax.device_put(a, NamedSharding(mesh, PartitionSpec()))
                _DEV_CACHE[n] = (key, dev, a)
                cached = _DEV_CACHE[n]
            ops.append(cached[1])
        outs = fn(*ops)
        y = np.asarray(outs[0]).reshape(B, N, C)
        return y.astype(np.float32)
    except Exception:
        from concourse import bass_utils

        nc = _get_nc()
        in_maps = [
            {
                "x": x[c],
                "Wqkv": Wqkv,
                "bqkv": bqkv,
                "Wproj": Wproj,
                "bproj": bproj,
            }
            for c in range(B)
        ]
        res = bass_utils.run_bass_kernel_spmd(nc, in_maps, core_ids=list(range(B)))
        return np.stack([res.results[c]["y"] for c in range(B)]).astype(np.float32)


# revision 39
# speedup vs baseline: 1.0221x; 1.0221x over previous
"""Multi-head self-attention Bass/Tile kernel for TRN2.

Per-core problem (batch sharded across 8 cores):
  x [N=1024, C=768], Wqkv [768, 2304], bqkv [2304], Wproj [768, 768], bproj [768]
  -> y [1024, 768]

Layout strategy (cost model charges matmuls by OUTPUT FREE SIZE only):
  xT   [C, N]      via PE transposes of x (bf16 identity -> 1 cyc/row)
  qkT  [2C, N]     = W.T-matmul with rhs xT (fp32r production), stored bf16
  V    [N, C]      xT-stationary matmul, stored bf16 padded per head with a
                   ones column (65 cols/head) for softmax sums
  S^T  [keys, n]   per head = kT.T @ qT  (K = 64, PE row groups alternate)
  P^T  = exp(S^T * 0.125)  (ACT, direct from PSUM, written bf16)
  O    [q, 65]     per query-tile = P^T-chunk.T @ [V_h | 1] accumulated over
                   key-tiles; col 64 = softmax sums.  Charged only 65 PE
                   rows/matmul (vs 512 in the [65, n] orientation).
  oc   [q, f]      = O[:, 0:64] * (1/O[:, 64]) per-partition scalars (DVE)
  aoT  [C, N]      = DMA crossbar transpose of oc (no PE/DVE cost)
  y    [N, C]      = aoT.T-matmul with rhs Wproj (bf16), accumulated into
                   SBUF in k-groups so most proj work overlaps attention

Scheduling: the attention inner loop is software-pipelined (PV for slot j-1
issued after S/exp of slot j) so the in-order PE queue never blocks on an
unfinished exp; qkT/V/proj production is split into fine-grained filler
closures pumped into the ~400ns PE gaps of the ACT-paced loop.
"""

import numpy as np

N = 1024
C = 768
H = 12
D = 64
NT = N // 128   # 8 token tiles
CT = C // 128   # 6 channel tiles
MT_QK = 2 * C // 128  # 12 feature tiles for q|k
PAIRS = H // 2  # 6
SCALE = 1.0 / np.sqrt(D)


def build_nc(mm_dtype_name="float32r"):
    import ml_dtypes
    import concourse.bass as bass
    import concourse.tile as tile
    from concourse import bacc, mybir
    from collections import deque

    mm_dt = getattr(mybir.dt, mm_dtype_name)
    f32 = mybir.dt.float32
    bf16 = mybir.dt.bfloat16

    nc = bacc.Bacc(None, target_bir_lowering=False)

    x = nc.dram_tensor("x", [N, C], f32, kind="ExternalInput")
    wqkv = nc.dram_tensor("Wqkv", [C, 3 * C], f32, kind="ExternalInput")
    bqkv = nc.dram_tensor("bqkv", [3 * C], f32, kind="ExternalInput")
    wproj = nc.dram_tensor("Wproj", [C, C], f32, kind="ExternalInput")
    bproj = nc.dram_tensor("bproj", [C], f32, kind="ExternalInput")
    y = nc.dram_tensor("y", [N, C], f32, kind="ExternalOutput")

    mm = nc.tensor.matmul

    with tile.TileContext(nc) as tc:
        with (
            tc.tile_pool(name="const", bufs=1) as const,
            tc.tile_pool(name="xt", bufs=1) as xt_pool,
            tc.tile_pool(name="qk", bufs=8) as qk_pool,
            tc.tile_pool(name="vpad", bufs=NT) as vpad_pool,
            tc.tile_pool(name="aot", bufs=CT) as aot_pool,
            tc.tile_pool(name="ocq", bufs=2) as oc_pool,
            tc.tile_pool(name="xin", bufs=4) as x_pool,
            tc.tile_pool(name="wqk", bufs=4) as wqk_pool,
            tc.tile_pool(name="wrhs", bufs=2 * CT) as wrhs_pool,
            tc.tile_pool(name="wstage", bufs=2) as wstage_pool,
            tc.tile_pool(name="pexp", bufs=12) as p_pool,
            tc.tile_pool(name="inv", bufs=2) as inv_pool,
            tc.tile_pool(name="yacc", bufs=NT) as yacc_pool,
            tc.tile_pool(name="yout", bufs=4) as y_pool,
            tc.tile_pool(name="psS", bufs=3, space="PSUM") as psS,
            tc.tile_pool(name="psO", bufs=2, space="PSUM") as psO,
        ):
            # ---- constants; ident first (transposes need it early) ----
            ident_dram = nc.inline_tensor(
                np.eye(128, dtype=ml_dtypes.bfloat16), name="ident128"
            )
            ident = const.tile([128, 128], bf16)
            nc.sync.dma_start(ident[:], ident_dram.ap())
            bq_cols = const.tile([128, MT_QK], f32)
            nc.gpsimd.dma_start(
                bq_cols[:], bqkv.ap().rearrange("(m p) -> p m", p=128)[:, 0:MT_QK]
            )
            bqv_row = const.tile([1, C], f32)
            nc.gpsimd.dma_start(bqv_row[:], bqkv.ap()[None, 2 * C : 3 * C])
            bqv_bc = const.tile([128, C], f32)
            nc.gpsimd.partition_broadcast(bqv_bc[:], bqv_row[:])
            bp_row = const.tile([1, C], f32)
            nc.gpsimd.dma_start(bp_row[:], bproj.ap()[None, :])
            bp_bc = const.tile([128, C], f32)
            nc.gpsimd.partition_broadcast(bp_bc[:], bp_row[:])
            ones_c = const.tile([128, H], f32)
            nc.vector.memset(ones_c[:], 1.0)

            # ---- upfront DMAs on the two HWDGE queues (SP + ACT); the
            # gpsimd/SWDGE queue is kept for consts + Wproj so x tiles are
            # never stuck behind slow Pool descriptor generation ----
            xi_t = []
            for i in range(NT):
                xi = x_pool.tile([128, C], f32, tag="xi", name=f"xi{i}")
                xi_t.append(xi)

            wqk_tiles = {}

            def wqk_dma(mi, eng):
                # one DMA per M-tile: [128, CT*128], col = k*128 + m
                w = wqk_pool.tile([128, C], mm_dt, name=f"wqk{mi}", tag="w")
                eng.dma_start(
                    w[:].bitcast(f32).rearrange("p (k m) -> p k m", k=CT),
                    wqkv.ap()[:, mi * 128 : (mi + 1) * 128].rearrange(
                        "(k p) m -> p k m", p=128
                    ),
                )
                wqk_tiles[mi] = w

            wv_tiles = [
                wrhs_pool.tile([128, C], mm_dt, name=f"wv{k}", tag="wrhs")
                for k in range(CT)
            ]

            def wv_dma(k, eng):
                eng.dma_start(
                    wv_tiles[k][:].bitcast(f32),
                    wqkv.ap()[k * 128 : (k + 1) * 128, 2 * C : 3 * C],
                )

            # arrival order on the shared DMA device tracks issue order:
            # x0-x3 + the two first-pair W tiles come first
            nc.sync.dma_start(xi_t[0][:], x.ap()[0:128, :])
            nc.scalar.dma_start(xi_t[1][:], x.ap()[128:256, :])
            nc.sync.dma_start(xi_t[2][:], x.ap()[256:384, :])
            nc.scalar.dma_start(xi_t[3][:], x.ap()[384:512, :])
            wqk_dma(0, nc.sync)
            wqk_dma(PAIRS, nc.scalar)
            nc.sync.dma_start(xi_t[4][:], x.ap()[512:640, :])
            nc.scalar.dma_start(xi_t[5][:], x.ap()[640:768, :])
            nc.sync.dma_start(xi_t[6][:], x.ap()[768:896, :])
            nc.scalar.dma_start(xi_t[7][:], x.ap()[896:1024, :])
            for k in range(CT):
                wv_dma(k, nc.sync if k % 2 == 0 else nc.scalar)

            # Wproj staged + cast to bf16 on Pool (needed from pair 3 on)
            wp_tiles = []
            for k in range(CT):
                stage = wstage_pool.tile([128, C], f32, name=f"wps{k}", tag="wstage")
                nc.gpsimd.dma_start(stage[:], wproj.ap()[k * 128 : (k + 1) * 128, :])
                wp = wrhs_pool.tile([128, C], bf16, name=f"wp{k}", tag="wrhs")
                nc.gpsimd.tensor_copy(wp[:], stage[:])
                wp_tiles.append(wp)

            # ---- xT via PE transposes (bf16 identity: 1 cyc/row) ----
            xt_all = xt_pool.tile([128, CT * N], mm_dt)
            xt3 = xt_all[:].rearrange("p (k n) -> p k n", k=CT)

            def xt_k(k):
                return xt_all[:, k * N : (k + 1) * N]

            for i in range(NT):
                pt = psS.tile([128, C], f32, tag="s", name=f"pt{i}")
                for j in range(CT):
                    nc.tensor.transpose(
                        pt[:, j * 128 : (j + 1) * 128].bitcast(mm_dt),
                        xi_t[i][:, j * 128 : (j + 1) * 128].bitcast(mm_dt),
                        ident[:],
                    )
                nc.vector.tensor_copy(
                    xt3[:, :, i * 128 : (i + 1) * 128],
                    pt[:].rearrange("p (k b) -> p k b", k=CT),
                )

            # ---- qkT production (bf16 storage, fp32r compute) ----
            qkT = [
                qk_pool.tile([128, N], bf16, name=f"qkT{m}", tag="qkT")
                for m in range(MT_QK)
            ]
            qk_ps = {}

            def qkT_mm(mi, c0, k):
                w = wqk_tiles[mi]
                if k == 0:
                    qk_ps[(mi, c0)] = psS.tile(
                        [128, 512], f32, tag="s", name=f"qps{mi}_{c0}"
                    )
                mm(qk_ps[(mi, c0)][:], w[:, k * 128 : (k + 1) * 128],
                   xt_k(k)[:, c0 : c0 + 512],
                   start=(k == 0), stop=(k == CT - 1))

            def qkT_evict(mi, c0):
                nc.vector.tensor_scalar_add(
                    qkT[mi][:, c0 : c0 + 512], qk_ps[(mi, c0)][:],
                    bq_cols[:, mi : mi + 1],
                )

            def qkT_chunk(mi, c0):
                for k in range(CT):
                    qkT_mm(mi, c0, k)
                qkT_evict(mi, c0)

            # pair-0 qkT inline, ordered so S(h0, j0) is ready earliest
            qkT_chunk(0, 0)
            qkT_chunk(PAIRS, 0)
            qkT_chunk(0, 512)
            qkT_chunk(PAIRS, 512)

            # ---- V (bf16, padded with ones col per head) ----
            vpad = [
                vpad_pool.tile([128, H * (D + 1)], bf16, name=f"vpad{t}", tag="vpad")
                for t in range(NT)
            ]
            v_ps = {}

            def V_part(ti, part):
                # parts 0-2: two k-steps each; part 3: bias add + ones col
                if part == 0:
                    v_ps[ti] = psS.tile([128, C], f32, tag="s", name=f"vps{ti}")
                ps = v_ps[ti]
                if part < 3:
                    for k in (2 * part, 2 * part + 1):
                        lhsT = xt_k(k)[:, ti * 128 : (ti + 1) * 128]
                        mm(ps[:, 0:512], lhsT, wv_tiles[k][:, 0:512],
                           start=(k == 0), stop=(k == CT - 1))
                        mm(ps[:, 512:768], lhsT, wv_tiles[k][:, 512:768],
                           start=(k == 0), stop=(k == CT - 1))
                else:
                    vsl = vpad[ti][:].rearrange("p (h d) -> p h d", h=H)
                    nc.vector.tensor_tensor(
                        out=vsl[:, :, 0:D],
                        in0=ps[:].rearrange("p (h d) -> p h d", h=H),
                        in1=bqv_bc[:].rearrange("p (h d) -> p h d", h=H),
                        op=mybir.AluOpType.add,
                    )
                    nc.vector.tensor_copy(vsl[:, :, D], ones_c[:])

            # ---- proj accumulators (SBUF) + k-group chains ----
            yacc = [
                yacc_pool.tile([128, C], f32, name=f"yacc{t}", tag="yacc")
                for t in range(NT)
            ]
            pj_ps = {}

            def proj_k(ti, k, first, last):
                if first:
                    pj_ps[ti] = psS.tile([128, C], f32, tag="s", name=f"pj{ti}_{k}")
                ps = pj_ps[ti]
                lhsT = aot[k][:, ti * 128 : (ti + 1) * 128]
                mm(ps[:, 0:512], lhsT, wp_tiles[k][:, 0:512],
                   start=first, stop=last)
                mm(ps[:, 512:768], lhsT, wp_tiles[k][:, 512:768],
                   start=first, stop=last)

            def proj_combine(ti, first):
                ps = pj_ps[ti]
                nc.vector.tensor_tensor(
                    out=yacc[ti][:],
                    in0=ps[:],
                    in1=(bp_bc[:] if first else yacc[ti][:]),
                    op=mybir.AluOpType.add,
                )

            # ---- attention structures ----
            aot = [
                aot_pool.tile([128, N], bf16, name=f"aot{j}", tag="aot")
                for j in range(CT)
            ]

            fill_q = deque()

            def pump(budget):
                while fill_q and budget > 0:
                    rows, fn = fill_q.popleft()
                    fn()
                    budget -= rows

            def enqueue_pair(pp):
                # weight DMAs issue immediately (a full pair of lead time);
                # only the fine-grained matmul/evict closures are pumped
                for mi in (pp, PAIRS + pp):
                    # pair 1 is enqueued at startup while gpsimd is clogged
                    # with consts/Wproj: use the SP HWDGE queue for both
                    eng = nc.sync if (mi == pp or pp == 1) else nc.gpsimd
                    wqk_dma(mi, eng)
                    for c0 in (0, 512):
                        for k in range(CT):
                            fill_q.append(
                                (512, lambda mi=mi, c0=c0, k=k: qkT_mm(mi, c0, k))
                            )
                        fill_q.append(
                            (0, lambda mi=mi, c0=c0: qkT_evict(mi, c0))
                        )

            def enqueue_proj(ks, first_k, tis):
                for ti in tis:
                    for idx, k in enumerate(ks):
                        fill_q.append(
                            (768,
                             lambda ti=ti, k=k, idx=idx, n=len(ks): proj_k(
                                 ti, k, idx == 0, idx == n - 1))
                        )
                    fill_q.append(
                        (0, lambda ti=ti, fk=first_k: proj_combine(ti, fk))
                    )

            enqueue_pair(1)

            # ---- software-pipelined attention loop ----
            # PV consumption lags S/exp issue by PV_LAG slots (one head), so
            # V production spreads over heads 0-1 instead of cramming into
            # head 0, and the in-order PE queue never blocks on an exp
            PV_LAG = 8
            pend_q = deque()
            oc_head = {}

            def issue_pv(h, j, P):
                if j == 0:
                    oc_head[h] = [
                        psO.tile([128, 512], f32, name=f"O{h}_{ci}", tag="o")
                        for ci in range(2)
                    ]
                Oc = oc_head[h]
                for qt in range(NT):
                    ci, q4 = divmod(qt, 4)
                    mm(Oc[ci][:, q4 * 65 : (q4 + 1) * 65],
                       P[:, qt * 128 : (qt + 1) * 128],
                       vpad[j][:, h * (D + 1) : (h + 1) * (D + 1)],
                       start=(j == 0 and q4 == 0),
                       stop=(j == NT - 1 and q4 == 3))
                if j == NT - 1:
                    normalize(h, Oc)

            def flush_pv(keep):
                while len(pend_q) > keep:
                    ph, pj, pP = pend_q.popleft()
                    if ph == 0:
                        v_issue_through(pj)
                    issue_pv(ph, pj, pP)

            # paced V production: all 32 part-closures must be issued before
            # the PV that consumes the corresponding vpad tile
            v_state = [0]

            def v_issue_through(ti):
                while v_state[0] < 4 * (ti + 1):
                    V_part(v_state[0] // 4, v_state[0] % 4)
                    v_state[0] += 1

            def v_pace(n):
                lim = min(4 * NT, v_state[0] + n)
                while v_state[0] < lim:
                    V_part(v_state[0] // 4, v_state[0] % 4)
                    v_state[0] += 1

            def normalize(h, Oc):
                # per-head half tile [q, qt*64+d]; transposed immediately so
                # only the last head's 448ns transpose sits in the tail
                p, h2 = divmod(h, 2)
                inv = inv_pool.tile([128, NT], f32, tag="inv", name=f"inv{h}")
                occ = oc_pool.tile([128, NT * D], bf16, name=f"oc{h}", tag="oc")
                for ci in range(2):
                    o3 = Oc[ci][:, 0:260].rearrange("p (q f) -> p q f", f=65)
                    nc.vector.reciprocal(inv[:, ci * 4 : (ci + 1) * 4], o3[:, :, D])
                    out3 = occ[:].rearrange("p (q f) -> p q f", f=D)
                    nc.vector.tensor_tensor(
                        out=out3[:, ci * 4 : (ci + 1) * 4, :],
                        in0=o3[:, :, 0:D],
                        in1=inv[:, ci * 4 : (ci + 1) * 4]
                        .unsqueeze(2)
                        .broadcast_to([128, 4, D]),
                        op=mybir.AluOpType.mult,
                    )
                # crossbar-transpose this head's 64 feature rows into aoT
                out3t = aot[p][h2 * D : (h2 + 1) * D, :].rearrange(
                    "p (q t) -> p q t", q=NT
                )
                if h == H - 1:
                    # final head: transpose each 4-qt half as soon as its
                    # normalize lands -- this is the tail's critical path
                    for ci in range(2):
                        nc.sync.dma_start_transpose(
                            out3t[:, ci * 4 : (ci + 1) * 4, :],
                            occ[:, ci * 4 * D : (ci + 1) * 4 * D],
                        )
                else:
                    nc.sync.dma_start_transpose(out3t, occ[:])

            for p in range(PAIRS):
                for h2 in range(2):
                    h = 2 * p + h2
                    hsl = slice(h2 * D, (h2 + 1) * D)
                    q_t = qkT[p]
                    k_t = qkT[PAIRS + p]
                    for j in range(NT):
                        S = psS.tile([128, N], f32, tag="s", name=f"S{h}_{j}")
                        mm(S[:, 0:512], k_t[hsl, j * 128 : (j + 1) * 128],
                           q_t[hsl, 0:512], start=True, stop=True,
                           tile_position=(h2 * D, 0))
                        mm(S[:, 512:1024], k_t[hsl, j * 128 : (j + 1) * 128],
                           q_t[hsl, 512:1024], start=True, stop=True,
                           tile_position=(h2 * D, 0))
                        P = p_pool.tile([128, N], bf16, tag="P", name=f"P{h}_{j}")
                        nc.scalar.activation(
                            P[:], S[:], mybir.ActivationFunctionType.Exp, scale=SCALE
                        )
                        # paced V production across the first two heads
                        if v_state[0] < 4 * NT:
                            v_pace(2)
                        pend_q.append((h, j, P))
                        flush_pv(PV_LAG)
                        if p == PAIRS - 1 and h2 == 1 and j == 1:
                            # aot[4]'s transpose has landed by now
                            enqueue_proj((4,), False, range(NT))
                        if h > 0:
                            pump(1024)
                # pair boundary: qkT for pair p+1 must be fully issued before
                # its first S matmul -> drain leftovers, then enqueue the next
                # batch of production work
                while fill_q:
                    fill_q.popleft()[1]()
                if p < PAIRS - 2:
                    enqueue_pair(p + 2)
                if p == 2:
                    # aot[0], aot[1] transposes landed during pair 2
                    enqueue_proj((0, 1), True, range(0, 4))
                elif p == 3:
                    enqueue_proj((0, 1), True, range(4, NT))
                    enqueue_proj((2, 3), False, range(0, 4))
                elif p == 4:
                    enqueue_proj((2, 3), False, range(4, NT))

            # ---- tail: drain pending PVs + final proj k-group ----
            flush_pv(0)
            for ti in range(NT):
                proj_k(ti, 5, True, True)
                ps = pj_ps[ti]
                yo = y_pool.tile([128, C], f32, tag="yo", name=f"yo{ti}")
                nc.vector.tensor_tensor(
                    out=yo[:], in0=ps[:], in1=yacc[ti][:], op=mybir.AluOpType.add
                )
                (nc.sync if ti % 2 == 0 else nc.scalar).dma_start(
                    y.ap()[ti * 128 : (ti + 1) * 128, :], yo[:]
                )

    nc.compile()
    return nc


_NC_CACHE = {}


def _get_nc(mm_dtype_name="float32r"):
    nc = _NC_CACHE.get(mm_dtype_name)
    if nc is None:
        nc = build_nc(mm_dtype_name)
        _NC_CACHE[mm_dtype_name] = nc
    return nc


_RUNNER_CACHE = {}
_DEV_CACHE = {}


def _get_runner(n_cores=8):
    """Cached jitted 8-core executor (PJRT path, no per-call retrace)."""
    if n_cores in _RUNNER_CACHE:
        return _RUNNER_CACHE[n_cores]
    import jax
    from jax.sharding import Mesh, PartitionSpec
    from jax.experimental.shard_map import shard_map
    from concourse import mybir
    from concourse.bass2jax import (
        _bass_exec_p,
        install_neuronx_cc_hook,
        partition_id_tensor,
    )

    nc = _get_nc()
    install_neuronx_cc_hook()
    partition_name = nc.partition_id_tensor.name if nc.partition_id_tensor else None

    in_names, out_names, out_avals = [], [], []
    for alloc in nc.m.functions[0].allocations:
        if not isinstance(alloc, mybir.MemoryLocationSet):
            continue
        name = alloc.memorylocations[0].name
        if alloc.kind == "ExternalInput":
            if name != partition_name:
                in_names.append(name)
        elif alloc.kind == "ExternalOutput":
            out_names.append(name)
            out_avals.append(
                jax.core.ShapedArray(
                    tuple(alloc.tensor_shape), mybir.dt.np(alloc.dtype)
                )
            )
    all_in_names = list(in_names)
    if partition_name is not None:
        all_in_names.append(partition_name)

    def _body(*args):
        operands = list(args)
        if partition_name is not None:
            operands.append(partition_id_tensor())
        return tuple(
            _bass_exec_p.bind(
                *operands,
                out_avals=tuple(out_avals),
                in_names=tuple(all_in_names),
                out_names=tuple(out_names),
                lowering_input_output_aliases=(),
                sim_require_finite=False,
                sim_require_nnan=False,
                nc=nc,
            )
        )

    devices = jax.devices()[:n_cores]
    mesh = Mesh(np.asarray(devices), ("core",))
    # x is batch-sharded; weights/biases are replicated (shipped once, not
    # 8x-concatenated on the host).
    in_specs = tuple(
        PartitionSpec("core") if n == "x" else PartitionSpec() for n in in_names
    )
    fn = jax.jit(
        shard_map(
            _body,
            mesh=mesh,
            in_specs=in_specs,
            out_specs=(PartitionSpec("core"),) * len(out_names),
            check_rep=False,
        ),
        keep_unused=True,
    )
    _RUNNER_CACHE[n_cores] = (fn, in_names, mesh)
    return _RUNNER_CACHE[n_cores]


def kernel(x, Wqkv, bqkv, Wproj, bproj):
    """Full-input entry point.

    x [8, 1024, 768] is sharded one batch element per NeuronCore (data
    parallel, weights replicated, no collectives); outputs are re-stacked.
    """
    x = np.ascontiguousarray(np.asarray(x, dtype=np.float32))
    Wqkv = np.ascontiguousarray(np.asarray(Wqkv, dtype=np.float32))
    bqkv = np.ascontiguousarray(np.asarray(bqkv, dtype=np.float32))
    Wproj = np.ascontiguousarray(np.asarray(Wproj, dtype=np.float32))
    bproj = np.ascontiguousarray(np.asarray(bproj, dtype=np.float32))
    B = x.shape[0]
    assert x.shape == (8, N, C), f"expected (8, {N}, {C}), got {x.shape}"

    arrays = {
        "x": x.reshape(B * N, C),
        "Wqkv": Wqkv,
        "bqkv": bqkv,
        "Wproj": Wproj,
        "bproj": bproj,
    }
    try:
        import jax
        from jax.sharding import NamedSharding, PartitionSpec

        fn, in_names, mesh = _get_runner(B)
        ops = []
        for n in in_names:
            a = arrays[n]
            if n == "x":
                ops.append(a)  # sharded fresh each call
                continue
            # weights rarely change call-to-call: keep them device-resident
            key = (n, id(a), a.shape)
            cached = _DEV_CACHE.get(n)
            if cached is None or cached[0] != key:
                dev = jax.device_put(a, NamedSharding(mesh, PartitionSpec()))
                _DEV_CACHE[n] = (key, dev, a)
                cached = _DEV_CACHE[n]
            ops.append(cached[1])
        outs = fn(*ops)
        y = np.asarray(outs[0]).reshape(B, N, C)
        return y.astype(np.float32)
    except Exception:
        from concourse import bass_utils

        nc = _get_nc()
        in_maps = [
            {
                "x": x[c],
                "Wqkv": Wqkv,
                "bqkv": bqkv,
                "Wproj": Wproj,
                "bproj": bproj,
            }
            for c in range(B)
        ]
        res = bass_utils.run_bass_kernel_spmd(nc, in_maps, core_ids=list(range(B)))
        return np.stack([res.results[c]["y"] for c in range(B)]).astype(np.float32)
